# revision 45
# baseline (speedup 1.0000x reference)
"""Trainium2 Bass kernel for nn_Attention (dense transformer attention block).

Reference semantics (B=2, S=2048, D=2048, NH=16, NKV=4, HD=128):
    qkv = x @ wqkv.T ; split q/k/v ; rmsnorm(q), rmsnorm(k) (weights == 1)
    rotary(q), rotary(k) with arbitrary freqs_cis ; GQA repeat kv 4x
    causal softmax attention ; out = y @ wo.T

Sharding: 2-way data parallel over batch x 4-way tensor parallel over head
groups (each core owns 4 query heads + their single shared KV head).  Each
core computes a full-shape partial of the output projection for its batch
element; the host sums the 4 partials per batch element (the "all-reduce").

Device layout notes:
  - All tensors are kept "transposed" (feature dim on partitions, tokens on
    the free dim) so every matmul chains without transposes; only V is
    transposed on-device (PE transpose) to the [token, dv] layout the
    attention-value matmul needs as its stationary operand.
  - Head dims are permuted (even dims then odd dims) on the host so rotary
    becomes two contiguous 64-partition slabs; the permutation cancels in
    the q.k contraction and V/wo are left in natural order.
  - Softmax uses no running max: |scores| <= sqrt(128)*max|f|^2 is far below
    exp overflow in fp32 (verified empirically), so exp() is applied
    directly and the denominator is accumulated with a ones-vector matmul.
"""

import math
from contextlib import ExitStack

import numpy as np

B, S, D = 2, 2048, 2048
NH, NKV, HD = 16, 4, 128
EPS = 1e-6
N_CORES = 8
TPC = 4            # tensor-parallel cores per batch element
HEADS_PER_CORE = NH // TPC          # 4
Q_SIZE, KV_SIZE = NH * HD, NKV * HD
E_LOC = HEADS_PER_CORE * HD         # 512 local y/e dims per core
TT = 512                            # token tile (free dim) for matmuls
N_TT = S // TT                      # 4
N_KT = D // 128                     # 16 contraction tiles for projections
N_SKT = S // 128                    # 16 key tiles per sequence

_F32 = "float32"


def _steer_act_tables():
    """Make Exp and Ln both resolve to the combined natural_log_exp table.

    bacc's insert_act_table_loads picks the first act-function set that
    contains each function, which puts Exp and Ln in different tables and
    costs a ~1.3us ACT table re-load on every rmsnorm <-> softmax switch
    (measured 40 loads / 51us per core).  Stripping Exp/Ln from the other
    sets (list positions preserved, so set ids stay valid for walrus)
    leaves one shared table and a single load.
    """
    from concourse import bacc
    import concourse.mybir as mybir
    import concourse.hw_specs as hw_specs

    if getattr(bacc.get_activation_tables, "_act_steered", False):
        return
    orig = hw_specs.get_activation_tables

    def steered(arch):
        tabs = orig(arch)
        for name, fns in tabs.items():
            if name != "natural_log_exp_and_others":
                fns.discard(mybir.ActivationFunctionType.Exp)
                fns.discard(mybir.ActivationFunctionType.Ln)
        return tabs

    steered._act_steered = True
    bacc.get_activation_tables = steered


def _build_bass():
    import concourse.bass as bass  # noqa: F401
    import concourse.mybir as mybir
    import concourse.tile as tile
    from concourse import bacc
    from concourse.masks import make_identity

    _steer_act_tables()

    f32 = mybir.dt.float32
    bf16 = mybir.dt.bfloat16

    nc = bacc.Bacc("TRN2", target_bir_lowering=False, debug=False,
                   num_devices=N_CORES)

    # ---- DRAM I/O (per-core shards supplied via in_maps) ----
    xT_d = nc.dram_tensor("xT", (D, S), bf16, kind="ExternalInput").ap()
    # per-chunk-contiguous weight layout: [chunk, p, ko, e] so each chunk's
    # stationary tiles stream in with 4 KiB/partition contiguous lines
    wqkvT_d = nc.dram_tensor(
        "wqkvT", (HEADS_PER_CORE + 2, 128, N_KT, HD), bf16,
        kind="ExternalInput").ap()
    woT_d = nc.dram_tensor("woT", (E_LOC, D), bf16, kind="ExternalInput").ap()
    # fr/fi are duplicated across both 64-partition halves so rotary ops can
    # pair them with either the even (base 0) or odd (base 64) slab of q/k
    fr_d = nc.dram_tensor("fr", (HD, S), f32, kind="ExternalInput").ap()
    fi_d = nc.dram_tensor("fi", (HD, S), f32, kind="ExternalInput").ap()
    mask_d = nc.dram_tensor("mask", (128, 4 * TT), bf16,
                            kind="ExternalInput").ap()
    outT_d = nc.dram_tensor("outT", (D, S), f32, kind="ExternalOutput").ap()

    NCHUNK = HEADS_PER_CORE + 2     # 4 q heads, 1 k head, 1 v head
    SCALE = 1.0 / math.sqrt(HD)

    with tile.TileContext(nc) as tc, ExitStack() as ctx:
        # ---------- pools ----------
        const = ctx.enter_context(tc.tile_pool(name="const", bufs=1))
        sb = ctx.enter_context(tc.tile_pool(name="sb", bufs=2))
        epool = ctx.enter_context(tc.tile_pool(name="epool", bufs=5))
        psum = ctx.enter_context(tc.tile_pool(name="psum", bufs=2,
                                              space="PSUM"))
        pproj = ctx.enter_context(tc.tile_pool(name="pproj", bufs=2,
                                               space="PSUM"))
        pacc = ctx.enter_context(tc.tile_pool(name="pacc", bufs=2,
                                              space="PSUM"))
        prow = ctx.enter_context(tc.tile_pool(name="prow", bufs=2,
                                              space="PSUM"))

        # ---------- resident tensors ----------
        # phase-1-only tensors live in their own pool, freed before attention
        # needs peak SBUF
        p1_ctx = ExitStack()
        p1 = p1_ctx.enter_context(tc.tile_pool(name="p1", bufs=1))
        p1w = p1_ctx.enter_context(tc.tile_pool(name="p1w", bufs=2))
        # DMA priority: the k-chunk weights + first xT tiles gate the first
        # matmul, so they go first; bulk/constant loads go on the gpsimd
        # DMA queue so their issue cost doesn't delay the critical loads.
        xT = p1.tile([128, N_KT, S], bf16)               # 64 KiB/part
        xT_r = xT_d.rearrange("(ko p) t -> p ko t", p=128)
        fr = p1.tile([HD, S], f32)
        fi = p1.tile([HD, S], f32)
        woT = const.tile([128, HEADS_PER_CORE, D], bf16)
        cmask = const.tile([128, 4 * TT], bf16)

        def load_wch(chunk):
            wch = p1w.tile([128, N_KT, HD], bf16, tag="wch", name="wch")
            nc.sync.dma_start(wch[:], wqkvT_d[chunk])
            return wch

        wch_next = load_wch(HEADS_PER_CORE)    # k-chunk weights first
        for kt in range(N_KT):
            # per-k-tile loads so the first matmuls start ~1/16 in
            nc.sync.dma_start(xT[:, kt], xT_r[:, kt])
        nc.gpsimd.dma_start(fr[:], fr_d)
        nc.gpsimd.dma_start(fi[:], fi_d)
        nc.gpsimd.dma_start(
            woT[:], woT_d.rearrange("(eo p) d -> p eo d", p=128))
        nc.gpsimd.dma_start(cmask[:], mask_d)

        ident = const.tile([128, 128], bf16)
        make_identity(nc, ident[:])
        ones = const.tile([128, 1], bf16)
        nc.vector.memset(ones[:], 1.0)
        epsb = const.tile([1, 1], f32)
        nc.vector.memset(epsb[:], EPS)

        # rotated q (4 heads), rotated k, and v in [token, dv] layout
        qrot = [const.tile([128, S], bf16, tag=f"qrot{h}", name=f"qrot{h}")
                for h in range(HEADS_PER_CORE)]
        krot = const.tile([128, S], bf16)
        vT = const.tile([128, S], bf16)
        vtok = const.tile([128, N_SKT, HD], bf16)
        # normalized attention outputs (yT), stationary input of out-proj
        yT = [const.tile([128, S], bf16, tag=f"yT{h}", name=f"yT{h}")
              for h in range(HEADS_PER_CORE)]

        # ---------- phase 1 helper: one projection chunk (+norm+rotary) ----
        def project_chunk(chunk, wch=None):
            is_v = chunk == HEADS_PER_CORE + 1
            is_k = chunk == HEADS_PER_CORE
            if wch is None:
                wch = load_wch(chunk)
            for tt in range(N_TT):
                ts = slice(tt * TT, (tt + 1) * TT)
                ps = pproj.tile([128, TT], f32, tag="proj", name="ps")
                for kt in range(N_KT):
                    nc.tensor.matmul(
                        ps[:], wch[:, kt, :],
                        xT[:, kt, ts], start=(kt == 0), stop=(kt == N_KT - 1))
                if is_v:
                    nc.vector.tensor_copy(vT[:, ts], ps[:])
                    continue
                # rms stats: mean over head dim (partitions) via ones-matmul;
                # square runs on ACT (same table set as exp/ln)
                sq = sb.tile([128, TT], bf16, tag="sq", name="sq")
                nc.scalar.activation(sq[:], ps[:],
                                     mybir.ActivationFunctionType.Square)
                ms = prow.tile([1, TT], f32, tag="row", name="ms")
                nc.tensor.matmul(ms[:], ones[:], sq[:], start=True, stop=True)
                lnms = sb.tile([1, TT], f32, tag="lnms", name="lnms")
                nc.scalar.activation(lnms[:], ms[:],
                                     mybir.ActivationFunctionType.Ln,
                                     bias=epsb[:], scale=1.0 / HD)
                rs = sb.tile([1, TT], f32, tag="rs", name="rs")
                nc.scalar.activation(rs[:], lnms[:],
                                     mybir.ActivationFunctionType.Exp,
                                     bias=0.0, scale=-0.5)
                rsb = sb.tile([128, TT], f32, tag="rsb", name="rsb")
                nc.gpsimd.partition_broadcast(rsb[:], rs[:])
                # rotary, even dims on partitions 0:64, odd on 64:128:
                #   a      = q * fr            (both halves at once)
                #   bswap  = swap_halves(q) * [+fi; -fi]  (2 cross-half muls;
                #            the sign baked into fi makes the combine an add)
                #   rot    = a + bswap
                rot = sb.tile([128, TT], f32, tag="rot", name="rot")
                a = sb.tile([128, TT], f32, tag="rota", name="a")
                nc.vector.tensor_mul(a[:], ps[:], fr[:, ts])
                bsw = sb.tile([128, TT], f32, tag="rotb", name="bsw")
                nc.vector.tensor_mul(bsw[0:64, :], ps[64:128, :],
                                     fi[64:128, ts])
                nc.vector.tensor_mul(bsw[64:128, :], ps[0:64, :],
                                     fi[0:64, ts])
                nc.vector.tensor_add(rot[:], a[:], bsw[:])
                dst = krot if is_k else qrot[chunk]
                nc.vector.tensor_mul(dst[:, ts], rot[:], rsb[:])

        # ---------- phase 2 helper: one attention unit (head h, tile qt) ---
        def attention_unit(h, qt):
            qs = slice(qt * TT, (qt + 1) * TT)
            ntk = 4 * (qt + 1)
            dps = prow.tile([1, TT], f32, tag="row", name="dps")
            yps = pacc.tile([128, TT], f32, tag="yacc", name="yps")
            for tk in range(ntk):
                sps = psum.tile([128, TT], f32, tag="mm", name="sps")
                nc.tensor.matmul(sps[:],
                                 krot[:, tk * 128:(tk + 1) * 128],
                                 qrot[h][:, qs], start=True, stop=True)
                e = epool.tile([128, TT], bf16, tag="e", name="e")
                nc.scalar.activation(e[:], sps[:],
                                     mybir.ActivationFunctionType.Exp,
                                     bias=0.0, scale=SCALE)
                r = tk - 4 * qt
                if r >= 0:
                    nc.vector.tensor_mul(
                        e[:], e[:], cmask[:, r * TT:(r + 1) * TT])
                nc.tensor.matmul(dps[:], ones[:], e[:],
                                 start=(tk == 0), stop=(tk == ntk - 1))
                nc.tensor.matmul(yps[:], vtok[:, tk, :], e[:],
                                 start=(tk == 0), stop=(tk == ntk - 1))
            dr = sb.tile([1, TT], f32, tag="dr", name="dr")
            nc.vector.reciprocal_approx_fast(dr[:], dps[:])
            drb = sb.tile([128, TT], f32, tag="drb", name="drb")
            nc.gpsimd.partition_broadcast(drb[:], dr[:])
            nc.vector.tensor_mul(yT[h][:, qs], yps[:], drb[:])

        # ---------- phase 3 helper: out-projection for one query tile ------
        def outproj(qt):
            qs = slice(qt * TT, (qt + 1) * TT)
            for m in range(D // 128):
                ops = pproj.tile([128, TT], f32, tag="proj", name="ops")
                for e in range(HEADS_PER_CORE):
                    nc.tensor.matmul(ops[:],
                                     woT[:, e, m * 128:(m + 1) * 128],
                                     yT[e][:, qs], start=(e == 0),
                                     stop=(e == HEADS_PER_CORE - 1))
                osb = sb.tile([128, TT], f32, tag="osb", name="osb")
                nc.vector.tensor_copy(osb[:], ops[:])
                nc.sync.dma_start(outT_d[m * 128:(m + 1) * 128, qs], osb[:])

        # ---------- emission: interleave attention with projection so PE
        # always has independent matmuls to issue while ACT runs exp -------
        project_chunk(HEADS_PER_CORE, wch=wch_next)    # k
        project_chunk(HEADS_PER_CORE + 1)              # v
        for i in range(N_SKT):                 # v -> [token, dv] layout
            tp = psum.tile([128, 128], bf16, tag="mm", name="tp")
            nc.tensor.transpose(tp[:], vT[:, i * 128:(i + 1) * 128], ident[:])
            nc.vector.tensor_copy(vtok[:, i, :], tp[:])
        for h in range(HEADS_PER_CORE):
            project_chunk(h)
        p1_ctx.close()   # xT/wqkvT/fr/fi no longer needed
        # largest query tiles first (their attention overlaps the remaining
        # projections); qt=3's out-projection is emitted last so its 64
        # dependency-free matmuls fill the kernel tail
        for qt in range(N_TT - 1, -1, -1):
            for h in range(HEADS_PER_CORE):
                attention_unit(h, qt)
            if qt < N_TT - 1:
                outproj(qt)
        outproj(N_TT - 1)

    nc.compile()
    return nc


def _host_shards(x, freqs_cis, wqkv, wo):
    import ml_dtypes
    bf16 = ml_dtypes.bfloat16

    # head-dim permutation: even dims then odd dims (for q and k only)
    perm = np.concatenate([np.arange(0, HD, 2), np.arange(1, HD, 2)])

    wq = wqkv[:Q_SIZE].reshape(NH, HD, D)[:, perm, :]
    wk = wqkv[Q_SIZE:Q_SIZE + KV_SIZE].reshape(NKV, HD, D)[:, perm, :]
    wv = wqkv[Q_SIZE + KV_SIZE:].reshape(NKV, HD, D)

    fr1 = np.ascontiguousarray(freqs_cis[:, :, 0].T, dtype=np.float32)
    fi1 = np.ascontiguousarray(freqs_cis[:, :, 1].T, dtype=np.float32)
    fr = np.vstack([fr1, fr1])
    # sign baked in so the rotary combine is a single add:
    #   rot[lo] = q_lo*fr + q_hi*(-fi) ; rot[hi] = q_hi*fr + q_lo*(+fi)
    fi = np.vstack([fi1, -fi1])

    # 4 diagonal-block causal masks for 128-row x 512-col score tiles
    tkl = np.arange(128)[:, None]
    tql = np.arange(TT)[None, :]
    mask = np.concatenate(
        [(tkl <= tql - 128 * r).astype(bf16) for r in range(4)], axis=1)

    in_maps = []
    for c in range(N_CORES):
        b, j = divmod(c, TPC)
        wshard = np.concatenate(
            [wq[TPC * j + h] for h in range(HEADS_PER_CORE)] +
            [wk[j], wv[j]], axis=0)                     # (768, D)
        # [chunk, p, ko, e] with d = ko*128 + p
        wpack = np.ascontiguousarray(
            wshard.reshape(HEADS_PER_CORE + 2, HD, N_KT, 128)
            .transpose(0, 3, 2, 1)).astype(bf16)
        in_maps.append({
            "xT": np.ascontiguousarray(x[b].T).astype(bf16),
            "wqkvT": wpack,
            "woT": np.ascontiguousarray(
                wo[:, j * E_LOC:(j + 1) * E_LOC].T).astype(bf16),
            "fr": fr,
            "fi": fi,
            "mask": mask,
        })
    return in_maps


_NC_CACHE = {}


def _get_nc():
    if "nc" not in _NC_CACHE:
        _NC_CACHE["nc"] = _build_bass()
    return _NC_CACHE["nc"]


def kernel(x, freqs_cis, wqkv, wo, q_norm_w, k_norm_w, _want_results=False):
    # q_norm_w / k_norm_w are all-ones per the problem spec; rmsnorm weight
    # multiply is the identity and is folded away.
    from concourse.bass_utils import run_bass_kernel_spmd

    nc = _get_nc()
    in_maps = _host_shards(np.asarray(x, np.float32),
                           np.asarray(freqs_cis, np.float32),
                           np.asarray(wqkv, np.float32),
                           np.asarray(wo, np.float32))
    res = run_bass_kernel_spmd(nc, in_maps, core_ids=list(range(N_CORES)))
    parts = [r["outT"] for r in res.results]
    out = np.empty((B, S, D), np.float32)
    for b in range(B):
        acc = parts[TPC * b].astype(np.float32).copy()
        for j in range(1, TPC):
            acc += parts[TPC * b + j]
        out[b] = acc.T
    if _want_results:
        return out, res
    return out


# revision 46
# speedup vs baseline: 1.0403x; 1.0403x over previous
"""Trainium2 Bass kernel for nn_Attention (dense transformer attention block).

Reference semantics (B=2, S=2048, D=2048, NH=16, NKV=4, HD=128):
    qkv = x @ wqkv.T ; split q/k/v ; rmsnorm(q), rmsnorm(k) (weights == 1)
    rotary(q), rotary(k) with arbitrary freqs_cis ; GQA repeat kv 4x
    causal softmax attention ; out = y @ wo.T

Sharding: 2-way data parallel over batch x 4-way tensor parallel over head
groups (each core owns 4 query heads + their single shared KV head).  Each
core computes a full-shape partial of the output projection for its batch
element; the host sums the 4 partials per batch element (the "all-reduce").

Device layout notes:
  - All tensors are kept "transposed" (feature dim on partitions, tokens on
    the free dim) so every matmul chains without transposes; only V is
    transposed on-device (PE transpose) to the [token, dv] layout the
    attention-value matmul needs as its stationary operand.
  - Head dims are permuted (even dims then odd dims) on the host so rotary
    becomes two contiguous 64-partition slabs; the permutation cancels in
    the q.k contraction and V/wo are left in natural order.
  - Softmax uses no running max: |scores| <= sqrt(128)*max|f|^2 is far below
    exp overflow in fp32 (verified empirically), so exp() is applied
    directly and the denominator is accumulated with a ones-vector matmul.
"""

import math
from contextlib import ExitStack

import numpy as np

B, S, D = 2, 2048, 2048
NH, NKV, HD = 16, 4, 128
EPS = 1e-6
N_CORES = 8
TPC = 4            # tensor-parallel cores per batch element
HEADS_PER_CORE = NH // TPC          # 4
Q_SIZE, KV_SIZE = NH * HD, NKV * HD
E_LOC = HEADS_PER_CORE * HD         # 512 local y/e dims per core
TT = 512                            # token tile (free dim) for matmuls
N_TT = S // TT                      # 4
N_KT = D // 128                     # 16 contraction tiles for projections
N_SKT = S // 128                    # 16 key tiles per sequence

_F32 = "float32"


def _steer_act_tables():
    """Make Exp and Ln both resolve to the combined natural_log_exp table.

    bacc's insert_act_table_loads picks the first act-function set that
    contains each function, which puts Exp and Ln in different tables and
    costs a ~1.3us ACT table re-load on every rmsnorm <-> softmax switch
    (measured 40 loads / 51us per core).  Stripping Exp/Ln from the other
    sets (list positions preserved, so set ids stay valid for walrus)
    leaves one shared table and a single load.
    """
    from concourse import bacc
    import concourse.mybir as mybir
    import concourse.hw_specs as hw_specs

    if getattr(bacc.get_activation_tables, "_act_steered", False):
        return
    orig = hw_specs.get_activation_tables

    def steered(arch):
        tabs = orig(arch)
        for name, fns in tabs.items():
            if name != "natural_log_exp_and_others":
                fns.discard(mybir.ActivationFunctionType.Exp)
                fns.discard(mybir.ActivationFunctionType.Ln)
        return tabs

    steered._act_steered = True
    bacc.get_activation_tables = steered


def _build_bass():
    import concourse.bass as bass  # noqa: F401
    import concourse.mybir as mybir
    import concourse.tile as tile
    from concourse import bacc
    from concourse.masks import make_identity

    _steer_act_tables()

    f32 = mybir.dt.float32
    bf16 = mybir.dt.bfloat16

    nc = bacc.Bacc("TRN2", target_bir_lowering=False, debug=False,
                   num_devices=N_CORES)

    # ---- DRAM I/O (per-core shards supplied via in_maps) ----
    xT_d = nc.dram_tensor("xT", (D, S), bf16, kind="ExternalInput").ap()
    # per-chunk-contiguous weight layout: [chunk, p, ko, e] so each chunk's
    # stationary tiles stream in with 4 KiB/partition contiguous lines
    wqkvT_d = nc.dram_tensor(
        "wqkvT", (HEADS_PER_CORE + 2, 128, N_KT, HD), bf16,
        kind="ExternalInput").ap()
    woT_d = nc.dram_tensor("woT", (E_LOC, D), bf16, kind="ExternalInput").ap()
    # fr/fi are duplicated across both 64-partition halves so rotary ops can
    # pair them with either the even (base 0) or odd (base 64) slab of q/k
    fr_d = nc.dram_tensor("fr", (HD, S), f32, kind="ExternalInput").ap()
    fi_d = nc.dram_tensor("fi", (HD, S), f32, kind="ExternalInput").ap()
    mask_d = nc.dram_tensor("mask", (128, 4 * TT), bf16,
                            kind="ExternalInput").ap()
    outT_d = nc.dram_tensor("outT", (D, S), f32, kind="ExternalOutput").ap()

    NCHUNK = HEADS_PER_CORE + 2     # 4 q heads, 1 k head, 1 v head
    SCALE = 1.0 / math.sqrt(HD)

    with tile.TileContext(nc) as tc, ExitStack() as ctx:
        # ---------- pools ----------
        const = ctx.enter_context(tc.tile_pool(name="const", bufs=1))
        sb = ctx.enter_context(tc.tile_pool(name="sb", bufs=2))
        epool = ctx.enter_context(tc.tile_pool(name="epool", bufs=5))
        psum = ctx.enter_context(tc.tile_pool(name="psum", bufs=2,
                                              space="PSUM"))
        pproj = ctx.enter_context(tc.tile_pool(name="pproj", bufs=2,
                                               space="PSUM"))
        pacc = ctx.enter_context(tc.tile_pool(name="pacc", bufs=2,
                                              space="PSUM"))
        prow = ctx.enter_context(tc.tile_pool(name="prow", bufs=2,
                                              space="PSUM"))

        # ---------- resident tensors ----------
        # phase-1-only tensors live in their own pool, freed before attention
        # needs peak SBUF
        p1_ctx = ExitStack()
        p1 = p1_ctx.enter_context(tc.tile_pool(name="p1", bufs=1))
        p1w = p1_ctx.enter_context(tc.tile_pool(name="p1w", bufs=2))
        # DMA priority: the k-chunk weights + first xT tiles gate the first
        # matmul, so they go first; bulk/constant loads go on the gpsimd
        # DMA queue so their issue cost doesn't delay the critical loads.
        xT = p1.tile([128, N_KT, S], bf16)               # 64 KiB/part
        xT_r = xT_d.rearrange("(ko p) t -> p ko t", p=128)
        fr = p1.tile([HD, S], f32)
        fi = p1.tile([HD, S], f32)
        woT = const.tile([128, HEADS_PER_CORE, D], bf16)
        cmask = const.tile([128, 4 * TT], bf16)

        def load_wch(chunk):
            wch = p1w.tile([128, N_KT, HD], bf16, tag="wch", name="wch")
            nc.sync.dma_start(wch[:], wqkvT_d[chunk])
            return wch

        wch_next = load_wch(HEADS_PER_CORE)    # k-chunk weights first
        for kt in range(N_KT):
            # per-k-tile loads so the first matmuls start ~1/16 in
            nc.sync.dma_start(xT[:, kt], xT_r[:, kt])
        nc.gpsimd.dma_start(fr[:], fr_d)
        nc.gpsimd.dma_start(fi[:], fi_d)
        nc.gpsimd.dma_start(
            woT[:], woT_d.rearrange("(eo p) d -> p eo d", p=128))
        nc.gpsimd.dma_start(cmask[:], mask_d)

        ident = const.tile([128, 128], bf16)
        make_identity(nc, ident[:])
        ones = const.tile([128, 1], bf16)
        nc.vector.memset(ones[:], 1.0)
        epsb = const.tile([1, 1], f32)
        nc.vector.memset(epsb[:], EPS)

        # rotated q (4 heads), rotated k, and v in [token, dv] layout
        qrot = [const.tile([128, S], bf16, tag=f"qrot{h}", name=f"qrot{h}")
                for h in range(HEADS_PER_CORE)]
        krot = const.tile([128, S], bf16)
        vT = const.tile([128, S], bf16)
        vtok = const.tile([128, N_SKT, HD], bf16)
        # normalized attention outputs (yT), stationary input of out-proj
        yT = [const.tile([128, S], bf16, tag=f"yT{h}", name=f"yT{h}")
              for h in range(HEADS_PER_CORE)]

        # ---------- phase 1 helper: one projection chunk (+norm+rotary) ----
        def project_chunk(chunk, wch=None):
            is_v = chunk == HEADS_PER_CORE + 1
            is_k = chunk == HEADS_PER_CORE
            if wch is None:
                wch = load_wch(chunk)
            for tt in range(N_TT):
                ts = slice(tt * TT, (tt + 1) * TT)
                ps = pproj.tile([128, TT], f32, tag="proj", name="ps")
                for kt in range(N_KT):
                    nc.tensor.matmul(
                        ps[:], wch[:, kt, :],
                        xT[:, kt, ts], start=(kt == 0), stop=(kt == N_KT - 1))
                if is_v:
                    nc.vector.tensor_copy(vT[:, ts], ps[:])
                    continue
                # rms stats: mean over head dim (partitions) via ones-matmul;
                # square runs on ACT (same table set as exp/ln)
                sq = sb.tile([128, TT], bf16, tag="sq", name="sq")
                nc.scalar.activation(sq[:], ps[:],
                                     mybir.ActivationFunctionType.Square)
                ms = prow.tile([1, TT], f32, tag="row", name="ms")
                nc.tensor.matmul(ms[:], ones[:], sq[:], start=True, stop=True)
                lnms = sb.tile([1, TT], f32, tag="lnms", name="lnms")
                nc.scalar.activation(lnms[:], ms[:],
                                     mybir.ActivationFunctionType.Ln,
                                     bias=epsb[:], scale=1.0 / HD)
                rs = sb.tile([1, TT], f32, tag="rs", name="rs")
                nc.scalar.activation(rs[:], lnms[:],
                                     mybir.ActivationFunctionType.Exp,
                                     bias=0.0, scale=-0.5)
                rsb = sb.tile([128, TT], f32, tag="rsb", name="rsb")
                nc.gpsimd.partition_broadcast(rsb[:], rs[:])
                # rotary, even dims on partitions 0:64, odd on 64:128:
                #   a      = q * fr            (both halves at once)
                #   bswap  = swap_halves(q) * [+fi; -fi]  (2 cross-half muls;
                #            the sign baked into fi makes the combine an add)
                #   rot    = a + bswap
                rot = sb.tile([128, TT], f32, tag="rot", name="rot")
                a = sb.tile([128, TT], f32, tag="rota", name="a")
                nc.vector.tensor_mul(a[:], ps[:], fr[:, ts])
                bsw = sb.tile([128, TT], f32, tag="rotb", name="bsw")
                nc.vector.tensor_mul(bsw[0:64, :], ps[64:128, :],
                                     fi[64:128, ts])
                nc.vector.tensor_mul(bsw[64:128, :], ps[0:64, :],
                                     fi[0:64, ts])
                nc.vector.tensor_add(rot[:], a[:], bsw[:])
                dst = krot if is_k else qrot[chunk]
                nc.vector.tensor_mul(dst[:, ts], rot[:], rsb[:])

        # ---------- phase 2 helper: one attention unit (head h, tile qt) ---
        def attention_unit(h, qt):
            qs = slice(qt * TT, (qt + 1) * TT)
            ntk = 4 * (qt + 1)
            dps = prow.tile([1, TT], f32, tag="row", name="dps")
            yps = pacc.tile([128, TT], f32, tag="yacc", name="yps")
            for tk in range(ntk):
                sps = psum.tile([128, TT], f32, tag="mm", name="sps")
                nc.tensor.matmul(sps[:],
                                 krot[:, tk * 128:(tk + 1) * 128],
                                 qrot[h][:, qs], start=True, stop=True)
                e = epool.tile([128, TT], bf16, tag="e", name="e")
                nc.scalar.activation(e[:], sps[:],
                                     mybir.ActivationFunctionType.Exp,
                                     bias=0.0, scale=SCALE)
                r = tk - 4 * qt
                if r >= 0:
                    nc.vector.tensor_mul(
                        e[:], e[:], cmask[:, r * TT:(r + 1) * TT])
                nc.tensor.matmul(dps[:], ones[:], e[:],
                                 start=(tk == 0), stop=(tk == ntk - 1))
                nc.tensor.matmul(yps[:], vtok[:, tk, :], e[:],
                                 start=(tk == 0), stop=(tk == ntk - 1))
            dr = sb.tile([1, TT], f32, tag="dr", name="dr")
            nc.vector.reciprocal_approx_fast(dr[:], dps[:])
            drb = sb.tile([128, TT], f32, tag="drb", name="drb")
            nc.gpsimd.partition_broadcast(drb[:], dr[:])
            nc.vector.tensor_mul(yT[h][:, qs], yps[:], drb[:])

        # ---------- phase 3 helper: out-projection for one query tile ------
        def outproj(qt):
            qs = slice(qt * TT, (qt + 1) * TT)
            for m in range(D // 128):
                ops = pproj.tile([128, TT], f32, tag="proj", name="ops")
                for e in range(HEADS_PER_CORE):
                    nc.tensor.matmul(ops[:],
                                     woT[:, e, m * 128:(m + 1) * 128],
                                     yT[e][:, qs], start=(e == 0),
                                     stop=(e == HEADS_PER_CORE - 1))
                osb = sb.tile([128, TT], f32, tag="osb", name="osb")
                nc.vector.tensor_copy(osb[:], ops[:])
                nc.sync.dma_start(outT_d[m * 128:(m + 1) * 128, qs], osb[:])

        # ---------- emission: interleave attention with projection so PE
        # always has independent matmuls to issue while ACT runs exp -------
        project_chunk(HEADS_PER_CORE, wch=wch_next)    # k
        project_chunk(HEADS_PER_CORE + 1)              # v
        for i in range(N_SKT):                 # v -> [token, dv] layout
            tp = psum.tile([128, 128], bf16, tag="mm", name="tp")
            nc.tensor.transpose(tp[:], vT[:, i * 128:(i + 1) * 128], ident[:])
            nc.vector.tensor_copy(vtok[:, i, :], tp[:])
        for h in range(HEADS_PER_CORE):
            project_chunk(h)
        p1_ctx.close()   # xT/wqkvT/fr/fi no longer needed
        # largest query tiles first: their attention overlaps the remaining
        # projections, and the kernel tail ends on the shortest chains
        for qt in range(N_TT - 1, -1, -1):
            for h in range(HEADS_PER_CORE):
                attention_unit(h, qt)
            # out-proj matmuls double as PE filler while ACT runs the next
            # tile's exps
            outproj(qt)

    nc.compile()
    return nc


def _host_shards(x, freqs_cis, wqkv, wo):
    import ml_dtypes
    bf16 = ml_dtypes.bfloat16

    # head-dim permutation: even dims then odd dims (for q and k only)
    perm = np.concatenate([np.arange(0, HD, 2), np.arange(1, HD, 2)])

    wq = wqkv[:Q_SIZE].reshape(NH, HD, D)[:, perm, :]
    wk = wqkv[Q_SIZE:Q_SIZE + KV_SIZE].reshape(NKV, HD, D)[:, perm, :]
    wv = wqkv[Q_SIZE + KV_SIZE:].reshape(NKV, HD, D)

    fr1 = np.ascontiguousarray(freqs_cis[:, :, 0].T, dtype=np.float32)
    fi1 = np.ascontiguousarray(freqs_cis[:, :, 1].T, dtype=np.float32)
    fr = np.vstack([fr1, fr1])
    # sign baked in so the rotary combine is a single add:
    #   rot[lo] = q_lo*fr + q_hi*(-fi) ; rot[hi] = q_hi*fr + q_lo*(+fi)
    fi = np.vstack([fi1, -fi1])

    # 4 diagonal-block causal masks for 128-row x 512-col score tiles
    tkl = np.arange(128)[:, None]
    tql = np.arange(TT)[None, :]
    mask = np.concatenate(
        [(tkl <= tql - 128 * r).astype(bf16) for r in range(4)], axis=1)

    in_maps = []
    for c in range(N_CORES):
        b, j = divmod(c, TPC)
        wshard = np.concatenate(
            [wq[TPC * j + h] for h in range(HEADS_PER_CORE)] +
            [wk[j], wv[j]], axis=0)                     # (768, D)
        # [chunk, p, ko, e] with d = ko*128 + p
        wpack = np.ascontiguousarray(
            wshard.reshape(HEADS_PER_CORE + 2, HD, N_KT, 128)
            .transpose(0, 3, 2, 1)).astype(bf16)
        in_maps.append({
            "xT": np.ascontiguousarray(x[b].T).astype(bf16),
            "wqkvT": wpack,
            "woT": np.ascontiguousarray(
                wo[:, j * E_LOC:(j + 1) * E_LOC].T).astype(bf16),
            "fr": fr,
            "fi": fi,
            "mask": mask,
        })
    return in_maps


_NC_CACHE = {}


def _get_nc():
    if "nc" not in _NC_CACHE:
        _NC_CACHE["nc"] = _build_bass()
    return _NC_CACHE["nc"]


def kernel(x, freqs_cis, wqkv, wo, q_norm_w, k_norm_w, _want_results=False):
    # q_norm_w / k_norm_w are all-ones per the problem spec; rmsnorm weight
    # multiply is the identity and is folded away.
    from concourse.bass_utils import run_bass_kernel_spmd

    nc = _get_nc()
    in_maps = _host_shards(np.asarray(x, np.float32),
                           np.asarray(freqs_cis, np.float32),
                           np.asarray(wqkv, np.float32),
                           np.asarray(wo, np.float32))
    res = run_bass_kernel_spmd(nc, in_maps, core_ids=list(range(N_CORES)))
    parts = [r["outT"] for r in res.results]
    out = np.empty((B, S, D), np.float32)
    for b in range(B):
        acc = parts[TPC * b].astype(np.float32).copy()
        for j in range(1, TPC):
            acc += parts[TPC * b + j]
        out[b] = acc.T
    if _want_results:
        return out, res
    return out


# revision 48
# speedup vs baseline: 1.0482x; 1.0076x over previous
"""Trainium2 Bass kernel for nn_Attention (dense transformer attention block).

Reference semantics (B=2, S=2048, D=2048, NH=16, NKV=4, HD=128):
    qkv = x @ wqkv.T ; split q/k/v ; rmsnorm(q), rmsnorm(k) (weights == 1)
    rotary(q), rotary(k) with arbitrary freqs_cis ; GQA repeat kv 4x
    causal softmax attention ; out = y @ wo.T

Sharding: 2-way data parallel over batch x 4-way tensor parallel over head
groups (each core owns 4 query heads + their single shared KV head).  Each
core computes a full-shape partial of the output projection for its batch
element; the host sums the 4 partials per batch element (the "all-reduce").

Device layout notes:
  - All tensors are kept "transposed" (feature dim on partitions, tokens on
    the free dim) so every matmul chains without transposes; only V is
    transposed on-device (PE transpose) to the [token, dv] layout the
    attention-value matmul needs as its stationary operand.
  - Head dims are permuted (even dims then odd dims) on the host so rotary
    becomes two contiguous 64-partition slabs; the permutation cancels in
    the q.k contraction and V/wo are left in natural order.
  - Softmax uses no running max: |scores| <= sqrt(128)*max|f|^2 is far below
    exp overflow in fp32 (verified empirically), so exp() is applied
    directly and the denominator is accumulated with a ones-vector matmul.
"""

import math
from contextlib import ExitStack

import numpy as np

B, S, D = 2, 2048, 2048
NH, NKV, HD = 16, 4, 128
EPS = 1e-6
N_CORES = 8
TPC = 4            # tensor-parallel cores per batch element
HEADS_PER_CORE = NH // TPC          # 4
Q_SIZE, KV_SIZE = NH * HD, NKV * HD
E_LOC = HEADS_PER_CORE * HD         # 512 local y/e dims per core
TT = 512                            # token tile (free dim) for matmuls
N_TT = S // TT                      # 4
N_KT = D // 128                     # 16 contraction tiles for projections
N_SKT = S // 128                    # 16 key tiles per sequence

_F32 = "float32"


def _steer_act_tables():
    """Make Exp and Ln both resolve to the combined natural_log_exp table.

    bacc's insert_act_table_loads picks the first act-function set that
    contains each function, which puts Exp and Ln in different tables and
    costs a ~1.3us ACT table re-load on every rmsnorm <-> softmax switch
    (measured 40 loads / 51us per core).  Stripping Exp/Ln from the other
    sets (list positions preserved, so set ids stay valid for walrus)
    leaves one shared table and a single load.
    """
    from concourse import bacc
    import concourse.mybir as mybir
    import concourse.hw_specs as hw_specs

    if getattr(bacc.get_activation_tables, "_act_steered", False):
        return
    orig = hw_specs.get_activation_tables

    def steered(arch):
        tabs = orig(arch)
        for name, fns in tabs.items():
            if name != "natural_log_exp_and_others":
                fns.discard(mybir.ActivationFunctionType.Exp)
                fns.discard(mybir.ActivationFunctionType.Ln)
        return tabs

    steered._act_steered = True
    bacc.get_activation_tables = steered


def _build_bass():
    import concourse.bass as bass  # noqa: F401
    import concourse.mybir as mybir
    import concourse.tile as tile
    from concourse import bacc
    from concourse.masks import make_identity

    _steer_act_tables()

    f32 = mybir.dt.float32
    bf16 = mybir.dt.bfloat16

    nc = bacc.Bacc("TRN2", target_bir_lowering=False, debug=False,
                   num_devices=N_CORES)

    # ---- DRAM I/O (per-core shards supplied via in_maps) ----
    xT_d = nc.dram_tensor("xT", (D, S), bf16, kind="ExternalInput").ap()
    # per-chunk-contiguous weight layout: [chunk, p, ko, e] so each chunk's
    # stationary tiles stream in with 4 KiB/partition contiguous lines
    wqkvT_d = nc.dram_tensor(
        "wqkvT", (HEADS_PER_CORE + 2, 128, N_KT, HD), bf16,
        kind="ExternalInput").ap()
    woT_d = nc.dram_tensor("woT", (E_LOC, D), bf16, kind="ExternalInput").ap()
    # fr/fi are duplicated across both 64-partition halves so rotary ops can
    # pair them with either the even (base 0) or odd (base 64) slab of q/k
    fr_d = nc.dram_tensor("fr", (HD, S), f32, kind="ExternalInput").ap()
    fi_d = nc.dram_tensor("fi", (HD, S), f32, kind="ExternalInput").ap()
    mask_d = nc.dram_tensor("mask", (128, 4 * TT), bf16,
                            kind="ExternalInput").ap()
    outT_d = nc.dram_tensor("outT", (D, S), f32, kind="ExternalOutput").ap()

    NCHUNK = HEADS_PER_CORE + 2     # 4 q heads, 1 k head, 1 v head
    SCALE = 1.0 / math.sqrt(HD)

    with tile.TileContext(nc) as tc, ExitStack() as ctx:
        # ---------- pools ----------
        const = ctx.enter_context(tc.tile_pool(name="const", bufs=1))
        sb = ctx.enter_context(tc.tile_pool(name="sb", bufs=2))
        epool = ctx.enter_context(tc.tile_pool(name="epool", bufs=5))
        psum = ctx.enter_context(tc.tile_pool(name="psum", bufs=2,
                                              space="PSUM"))
        pproj = ctx.enter_context(tc.tile_pool(name="pproj", bufs=2,
                                               space="PSUM"))
        pacc = ctx.enter_context(tc.tile_pool(name="pacc", bufs=2,
                                              space="PSUM"))
        prow = ctx.enter_context(tc.tile_pool(name="prow", bufs=2,
                                              space="PSUM"))

        # ---------- resident tensors ----------
        # phase-1-only tensors live in their own pool, freed before attention
        # needs peak SBUF
        p1_ctx = ExitStack()
        p1 = p1_ctx.enter_context(tc.tile_pool(name="p1", bufs=1))
        p1w = p1_ctx.enter_context(tc.tile_pool(name="p1w", bufs=2))
        # DMA priority: the k-chunk weights + first xT tiles gate the first
        # matmul, so they go first; bulk/constant loads go on the gpsimd
        # DMA queue so their issue cost doesn't delay the critical loads.
        xT = p1.tile([128, N_KT, S], bf16)               # 64 KiB/part
        xT_r = xT_d.rearrange("(ko p) t -> p ko t", p=128)
        fr = p1.tile([HD, S], f32)
        fi = p1.tile([HD, S], f32)
        woT = const.tile([128, HEADS_PER_CORE, D], bf16)
        cmask = const.tile([128, 4 * TT], bf16)

        def load_wch(chunk):
            wch = p1w.tile([128, N_KT, HD], bf16, tag="wch", name="wch")
            nc.sync.dma_start(wch[:], wqkvT_d[chunk])
            return wch

        wch_next = load_wch(HEADS_PER_CORE)    # k-chunk weights first
        for half in range(2):
            hs = slice(half * (S // 2), (half + 1) * (S // 2))
            for kt in range(N_KT):
                # stream x in the order the projection consumes it: all k
                # tiles of the first token half, then the second half
                nc.sync.dma_start(xT[:, kt, hs], xT_r[:, kt, hs])
        nc.gpsimd.dma_start(fr[:], fr_d)
        nc.gpsimd.dma_start(fi[:], fi_d)
        nc.gpsimd.dma_start(
            woT[:], woT_d.rearrange("(eo p) d -> p eo d", p=128))
        nc.gpsimd.dma_start(cmask[:], mask_d)

        ident = const.tile([128, 128], bf16)
        make_identity(nc, ident[:])
        ones = const.tile([128, 1], bf16)
        nc.vector.memset(ones[:], 1.0)
        epsb = const.tile([1, 1], f32)
        nc.vector.memset(epsb[:], EPS)

        # rotated q (4 heads), rotated k, and v in [token, dv] layout
        qrot = [const.tile([128, S], bf16, tag=f"qrot{h}", name=f"qrot{h}")
                for h in range(HEADS_PER_CORE)]
        krot = const.tile([128, S], bf16)
        vT = const.tile([128, S], bf16)
        vtok = const.tile([128, N_SKT, HD], bf16)
        # normalized attention outputs (yT), stationary input of out-proj
        yT = [const.tile([128, S], bf16, tag=f"yT{h}", name=f"yT{h}")
              for h in range(HEADS_PER_CORE)]

        # ---------- phase 1 helper: one projection chunk (+norm+rotary) ----
        def project_chunk(chunk, wch=None):
            is_v = chunk == HEADS_PER_CORE + 1
            is_k = chunk == HEADS_PER_CORE
            if wch is None:
                wch = load_wch(chunk)
            for tt in range(N_TT):
                ts = slice(tt * TT, (tt + 1) * TT)
                ps = pproj.tile([128, TT], f32, tag="proj", name="ps")
                for kt in range(N_KT):
                    nc.tensor.matmul(
                        ps[:], wch[:, kt, :],
                        xT[:, kt, ts], start=(kt == 0), stop=(kt == N_KT - 1))
                if is_v:
                    nc.vector.tensor_copy(vT[:, ts], ps[:])
                    continue
                # rms stats: mean over head dim (partitions) via ones-matmul;
                # square runs on ACT (same table set as exp/ln)
                sq = sb.tile([128, TT], bf16, tag="sq", name="sq")
                nc.scalar.activation(sq[:], ps[:],
                                     mybir.ActivationFunctionType.Square)
                ms = prow.tile([1, TT], f32, tag="row", name="ms")
                nc.tensor.matmul(ms[:], ones[:], sq[:], start=True, stop=True)
                lnms = sb.tile([1, TT], f32, tag="lnms", name="lnms")
                nc.scalar.activation(lnms[:], ms[:],
                                     mybir.ActivationFunctionType.Ln,
                                     bias=epsb[:], scale=1.0 / HD)
                rs = sb.tile([1, TT], f32, tag="rs", name="rs")
                nc.scalar.activation(rs[:], lnms[:],
                                     mybir.ActivationFunctionType.Exp,
                                     bias=0.0, scale=-0.5)
                rsb = sb.tile([128, TT], f32, tag="rsb", name="rsb")
                nc.gpsimd.partition_broadcast(rsb[:], rs[:])
                # rotary, even dims on partitions 0:64, odd on 64:128:
                #   a      = q * fr            (both halves at once)
                #   bswap  = swap_halves(q) * [+fi; -fi]  (2 cross-half muls;
                #            the sign baked into fi makes the combine an add)
                #   rot    = a + bswap
                rot = sb.tile([128, TT], f32, tag="rot", name="rot")
                a = sb.tile([128, TT], f32, tag="rota", name="a")
                nc.vector.tensor_mul(a[:], ps[:], fr[:, ts])
                bsw = sb.tile([128, TT], f32, tag="rotb", name="bsw")
                nc.vector.tensor_mul(bsw[0:64, :], ps[64:128, :],
                                     fi[64:128, ts])
                nc.vector.tensor_mul(bsw[64:128, :], ps[0:64, :],
                                     fi[0:64, ts])
                nc.vector.tensor_add(rot[:], a[:], bsw[:])
                dst = krot if is_k else qrot[chunk]
                nc.vector.tensor_mul(dst[:, ts], rot[:], rsb[:])

        # ---------- phase 2 helper: one attention unit (head h, tile qt) ---
        def attention_unit(h, qt):
            qs = slice(qt * TT, (qt + 1) * TT)
            ntk = 4 * (qt + 1)
            dps = prow.tile([1, TT], f32, tag="row", name="dps")
            yps = pacc.tile([128, TT], f32, tag="yacc", name="yps")
            for tk in range(ntk):
                sps = psum.tile([128, TT], f32, tag="mm", name="sps")
                nc.tensor.matmul(sps[:],
                                 krot[:, tk * 128:(tk + 1) * 128],
                                 qrot[h][:, qs], start=True, stop=True)
                e = epool.tile([128, TT], bf16, tag="e", name="e")
                nc.scalar.activation(e[:], sps[:],
                                     mybir.ActivationFunctionType.Exp,
                                     bias=0.0, scale=SCALE)
                r = tk - 4 * qt
                if r >= 0:
                    nc.vector.tensor_mul(
                        e[:], e[:], cmask[:, r * TT:(r + 1) * TT])
                nc.tensor.matmul(dps[:], ones[:], e[:],
                                 start=(tk == 0), stop=(tk == ntk - 1))
                nc.tensor.matmul(yps[:], vtok[:, tk, :], e[:],
                                 start=(tk == 0), stop=(tk == ntk - 1))
            dr = sb.tile([1, TT], f32, tag="dr", name="dr")
            nc.vector.reciprocal_approx_fast(dr[:], dps[:])
            drb = sb.tile([128, TT], f32, tag="drb", name="drb")
            nc.gpsimd.partition_broadcast(drb[:], dr[:])
            nc.vector.tensor_mul(yT[h][:, qs], yps[:], drb[:])

        # ---------- phase 3 helper: out-projection for one query tile ------
        def outproj(qt):
            qs = slice(qt * TT, (qt + 1) * TT)
            for m in range(D // 128):
                ops = pproj.tile([128, TT], f32, tag="proj", name="ops")
                for e in range(HEADS_PER_CORE):
                    nc.tensor.matmul(ops[:],
                                     woT[:, e, m * 128:(m + 1) * 128],
                                     yT[e][:, qs], start=(e == 0),
                                     stop=(e == HEADS_PER_CORE - 1))
                osb = sb.tile([128, TT], f32, tag="osb", name="osb")
                nc.vector.tensor_copy(osb[:], ops[:])
                nc.sync.dma_start(outT_d[m * 128:(m + 1) * 128, qs], osb[:])

        # ---------- emission: interleave attention with projection so PE
        # always has independent matmuls to issue while ACT runs exp -------
        project_chunk(HEADS_PER_CORE, wch=wch_next)    # k
        project_chunk(HEADS_PER_CORE + 1)              # v
        for i in range(N_SKT):                 # v -> [token, dv] layout
            tp = psum.tile([128, 128], bf16, tag="mm", name="tp")
            nc.tensor.transpose(tp[:], vT[:, i * 128:(i + 1) * 128], ident[:])
            nc.vector.tensor_copy(vtok[:, i, :], tp[:])
        for h in range(HEADS_PER_CORE):
            project_chunk(h)
        p1_ctx.close()   # xT/wqkvT/fr/fi no longer needed
        # largest query tiles first: their attention overlaps the remaining
        # projections.  The two smallest tiles are interleaved head-by-head
        # so the kernel tail has twice the independent chains to pipeline.
        for qt in (3, 2):
            for h in range(HEADS_PER_CORE):
                attention_unit(h, qt)
            # out-proj matmuls double as PE filler while ACT runs the next
            # tile's exps
            outproj(qt)
        for h in range(HEADS_PER_CORE):
            attention_unit(h, 1)
            attention_unit(h, 0)
        outproj(1)
        outproj(0)

    nc.compile()
    return nc


def _host_shards(x, freqs_cis, wqkv, wo):
    import ml_dtypes
    bf16 = ml_dtypes.bfloat16

    # head-dim permutation: even dims then odd dims (for q and k only)
    perm = np.concatenate([np.arange(0, HD, 2), np.arange(1, HD, 2)])

    wq = wqkv[:Q_SIZE].reshape(NH, HD, D)[:, perm, :]
    wk = wqkv[Q_SIZE:Q_SIZE + KV_SIZE].reshape(NKV, HD, D)[:, perm, :]
    wv = wqkv[Q_SIZE + KV_SIZE:].reshape(NKV, HD, D)

    fr1 = np.ascontiguousarray(freqs_cis[:, :, 0].T, dtype=np.float32)
    fi1 = np.ascontiguousarray(freqs_cis[:, :, 1].T, dtype=np.float32)
    fr = np.vstack([fr1, fr1])
    # sign baked in so the rotary combine is a single add:
    #   rot[lo] = q_lo*fr + q_hi*(-fi) ; rot[hi] = q_hi*fr + q_lo*(+fi)
    fi = np.vstack([fi1, -fi1])

    # 4 diagonal-block causal masks for 128-row x 512-col score tiles
    tkl = np.arange(128)[:, None]
    tql = np.arange(TT)[None, :]
    mask = np.concatenate(
        [(tkl <= tql - 128 * r).astype(bf16) for r in range(4)], axis=1)

    in_maps = []
    for c in range(N_CORES):
        b, j = divmod(c, TPC)
        wshard = np.concatenate(
            [wq[TPC * j + h] for h in range(HEADS_PER_CORE)] +
            [wk[j], wv[j]], axis=0)                     # (768, D)
        # [chunk, p, ko, e] with d = ko*128 + p
        wpack = np.ascontiguousarray(
            wshard.reshape(HEADS_PER_CORE + 2, HD, N_KT, 128)
            .transpose(0, 3, 2, 1)).astype(bf16)
        in_maps.append({
            "xT": np.ascontiguousarray(x[b].T).astype(bf16),
            "wqkvT": wpack,
            "woT": np.ascontiguousarray(
                wo[:, j * E_LOC:(j + 1) * E_LOC].T).astype(bf16),
            "fr": fr,
            "fi": fi,
            "mask": mask,
        })
    return in_maps


_NC_CACHE = {}


def _get_nc():
    if "nc" not in _NC_CACHE:
        _NC_CACHE["nc"] = _build_bass()
    return _NC_CACHE["nc"]


def kernel(x, freqs_cis, wqkv, wo, q_norm_w, k_norm_w, _want_results=False):
    # q_norm_w / k_norm_w are all-ones per the problem spec; rmsnorm weight
    # multiply is the identity and is folded away.
    from concourse.bass_utils import run_bass_kernel_spmd

    nc = _get_nc()
    in_maps = _host_shards(np.asarray(x, np.float32),
                           np.asarray(freqs_cis, np.float32),
                           np.asarray(wqkv, np.float32),
                           np.asarray(wo, np.float32))
    res = run_bass_kernel_spmd(nc, in_maps, core_ids=list(range(N_CORES)))
    parts = [r["outT"] for r in res.results]
    out = np.empty((B, S, D), np.float32)
    for b in range(B):
        acc = parts[TPC * b].astype(np.float32).copy()
        for j in range(1, TPC):
            acc += parts[TPC * b + j]
        out[b] = acc.T
    if _want_results:
        return out, res
    return out


# revision 52
# speedup vs baseline: 1.0709x; 1.0216x over previous
"""Trainium2 Bass kernel for nn_Attention (dense transformer attention block).

Reference semantics (B=2, S=2048, D=2048, NH=16, NKV=4, HD=128):
    qkv = x @ wqkv.T ; split q/k/v ; rmsnorm(q), rmsnorm(k) (weights == 1)
    rotary(q), rotary(k) with arbitrary freqs_cis ; GQA repeat kv 4x
    causal softmax attention ; out = y @ wo.T

Sharding: 2-way data parallel over batch x 4-way tensor parallel over head
groups (each core owns 4 query heads + their single shared KV head).  Each
core computes a full-shape partial of the output projection for its batch
element; the host sums the 4 partials per batch element (the "all-reduce").

Device layout notes:
  - All tensors are kept "transposed" (feature dim on partitions, tokens on
    the free dim) so every matmul chains without transposes; only V is
    transposed on-device (PE transpose) to the [token, dv] layout the
    attention-value matmul needs as its stationary operand.
  - Head dims are permuted (even dims then odd dims) on the host so rotary
    becomes two contiguous 64-partition slabs; the permutation cancels in
    the q.k contraction and V/wo are left in natural order.
  - Softmax uses no running max: |scores| <= sqrt(128)*max|f|^2 is far below
    exp overflow in fp32 (verified empirically), so exp() is applied
    directly and the denominator is accumulated with a ones-vector matmul.
"""

import math
from contextlib import ExitStack

import numpy as np

B, S, D = 2, 2048, 2048
NH, NKV, HD = 16, 4, 128
EPS = 1e-6
N_CORES = 8
TPC = 4            # tensor-parallel cores per batch element
HEADS_PER_CORE = NH // TPC          # 4
Q_SIZE, KV_SIZE = NH * HD, NKV * HD
E_LOC = HEADS_PER_CORE * HD         # 512 local y/e dims per core
TT = 512                            # token tile (free dim) for matmuls
N_TT = S // TT                      # 4
N_KT = D // 128                     # 16 contraction tiles for projections
N_SKT = S // 128                    # 16 key tiles per sequence

_F32 = "float32"


def _steer_act_tables():
    """Make Exp and Ln both resolve to the combined natural_log_exp table.

    bacc's insert_act_table_loads picks the first act-function set that
    contains each function, which puts Exp and Ln in different tables and
    costs a ~1.3us ACT table re-load on every rmsnorm <-> softmax switch
    (measured 40 loads / 51us per core).  Stripping Exp/Ln from the other
    sets (list positions preserved, so set ids stay valid for walrus)
    leaves one shared table and a single load.
    """
    from concourse import bacc
    import concourse.mybir as mybir
    import concourse.hw_specs as hw_specs

    if getattr(bacc.get_activation_tables, "_act_steered", False):
        return
    orig = hw_specs.get_activation_tables

    def steered(arch):
        tabs = orig(arch)
        for name, fns in tabs.items():
            if name != "natural_log_exp_and_others":
                fns.discard(mybir.ActivationFunctionType.Exp)
                fns.discard(mybir.ActivationFunctionType.Ln)
        return tabs

    steered._act_steered = True
    bacc.get_activation_tables = steered


def _build_bass():
    import concourse.bass as bass  # noqa: F401
    import concourse.mybir as mybir
    import concourse.tile as tile
    from concourse import bacc
    from concourse.masks import make_identity

    _steer_act_tables()

    f32 = mybir.dt.float32
    bf16 = mybir.dt.bfloat16

    nc = bacc.Bacc("TRN2", target_bir_lowering=False, debug=False,
                   num_devices=N_CORES)

    # ---- DRAM I/O (per-core shards supplied via in_maps) ----
    xT_d = nc.dram_tensor("xT", (D, S), bf16, kind="ExternalInput").ap()
    # per-chunk-contiguous weight layout: [chunk, p, ko, e] so each chunk's
    # stationary tiles stream in with 4 KiB/partition contiguous lines
    wqkvT_d = nc.dram_tensor(
        "wqkvT", (HEADS_PER_CORE + 2, 128, N_KT, HD), bf16,
        kind="ExternalInput").ap()
    woT_d = nc.dram_tensor("woT", (E_LOC, D), bf16, kind="ExternalInput").ap()
    # fr/fi are duplicated across both 64-partition halves so rotary ops can
    # pair them with either the even (base 0) or odd (base 64) slab of q/k
    fr_d = nc.dram_tensor("fr", (HD, S), f32, kind="ExternalInput").ap()
    fi_d = nc.dram_tensor("fi", (HD, S), f32, kind="ExternalInput").ap()
    mask_d = nc.dram_tensor("mask", (128, TT), bf16,
                            kind="ExternalInput").ap()
    outT_d = nc.dram_tensor("outT", (D, S), f32, kind="ExternalOutput").ap()

    NCHUNK = HEADS_PER_CORE + 2     # 4 q heads, 1 k head, 1 v head
    SCALE = 1.0 / math.sqrt(HD)

    with tile.TileContext(nc) as tc, ExitStack() as ctx:
        # ---------- pools ----------
        const = ctx.enter_context(tc.tile_pool(name="const", bufs=1))
        sb = ctx.enter_context(tc.tile_pool(name="sb", bufs=2))
        epool = ctx.enter_context(tc.tile_pool(name="epool", bufs=5))
        psum = ctx.enter_context(tc.tile_pool(name="psum", bufs=2,
                                              space="PSUM"))
        pproj = ctx.enter_context(tc.tile_pool(name="pproj", bufs=2,
                                               space="PSUM"))
        pacc = ctx.enter_context(tc.tile_pool(name="pacc", bufs=2,
                                              space="PSUM"))
        prow = ctx.enter_context(tc.tile_pool(name="prow", bufs=2,
                                              space="PSUM"))

        # ---------- resident tensors ----------
        # phase-1-only tensors live in their own pool, freed before attention
        # needs peak SBUF
        p1_ctx = ExitStack()
        p1 = p1_ctx.enter_context(tc.tile_pool(name="p1", bufs=1))
        p1w = p1_ctx.enter_context(tc.tile_pool(name="p1w", bufs=2))
        # DMA priority: the k-chunk weights + first xT tiles gate the first
        # matmul, so they go first; bulk/constant loads go on the gpsimd
        # DMA queue so their issue cost doesn't delay the critical loads.
        xT = p1.tile([128, N_KT, S], bf16)               # 64 KiB/part
        xT_r = xT_d.rearrange("(ko p) t -> p ko t", p=128)
        fr = p1.tile([HD, S], f32)
        fi = p1.tile([HD, S], f32)
        woT = const.tile([128, HEADS_PER_CORE, D], bf16)
        cmask = const.tile([128, TT], bf16)

        def load_wch(chunk):
            wch = p1w.tile([128, N_KT, HD], bf16, tag="wch", name="wch")
            nc.sync.dma_start(wch[:], wqkvT_d[chunk])
            return wch

        wch_next = load_wch(HEADS_PER_CORE)    # k-chunk weights first
        for half in range(2):
            hs = slice(half * (S // 2), (half + 1) * (S // 2))
            for kt in range(N_KT):
                # stream x in the order the projection consumes it: all k
                # tiles of the first token half, then the second half
                nc.sync.dma_start(xT[:, kt, hs], xT_r[:, kt, hs])
        nc.gpsimd.dma_start(fr[:], fr_d)
        nc.gpsimd.dma_start(fi[:], fi_d)
        nc.gpsimd.dma_start(
            woT[:], woT_d.rearrange("(eo p) d -> p eo d", p=128))
        nc.gpsimd.dma_start(cmask[:], mask_d)

        ident = const.tile([128, 128], bf16)
        make_identity(nc, ident[:])
        ones = const.tile([128, 1], bf16)
        nc.vector.memset(ones[:], 1.0)
        epsb = const.tile([1, 1], f32)
        nc.vector.memset(epsb[:], EPS)

        # rotated q (4 heads), rotated k, and v in [token, dv] layout
        qrot = [const.tile([128, S], bf16, tag=f"qrot{h}", name=f"qrot{h}")
                for h in range(HEADS_PER_CORE)]
        krot = const.tile([128, S], bf16)
        vT = const.tile([128, S], bf16)
        vtok = const.tile([128, N_SKT, HD], bf16)
        # normalized attention outputs (yT), stationary input of out-proj
        yT = [const.tile([128, S], bf16, tag=f"yT{h}", name=f"yT{h}")
              for h in range(HEADS_PER_CORE)]

        # ---------- phase 1 helper: one projection chunk (+norm+rotary) ----
        def project_chunk(chunk, wch=None):
            is_v = chunk == HEADS_PER_CORE + 1
            is_k = chunk == HEADS_PER_CORE
            if wch is None:
                wch = load_wch(chunk)
            for tt in range(N_TT):
                ts = slice(tt * TT, (tt + 1) * TT)
                ps = pproj.tile([128, TT], f32, tag="proj", name="ps")
                for kt in range(N_KT):
                    nc.tensor.matmul(
                        ps[:], wch[:, kt, :],
                        xT[:, kt, ts], start=(kt == 0), stop=(kt == N_KT - 1))
                if is_v:
                    nc.vector.tensor_copy(vT[:, ts], ps[:])
                    continue
                # rms stats: mean over head dim (partitions) via ones-matmul;
                # square runs on ACT (same table set as exp/ln)
                sq = sb.tile([128, TT], bf16, tag="sq", name="sq")
                nc.scalar.activation(sq[:], ps[:],
                                     mybir.ActivationFunctionType.Square)
                ms = prow.tile([1, TT], f32, tag="row", name="ms")
                nc.tensor.matmul(ms[:], ones[:], sq[:], start=True, stop=True)
                lnms = sb.tile([1, TT], f32, tag="lnms", name="lnms")
                nc.scalar.activation(lnms[:], ms[:],
                                     mybir.ActivationFunctionType.Ln,
                                     bias=epsb[:], scale=1.0 / HD)
                rs = sb.tile([1, TT], f32, tag="rs", name="rs")
                nc.scalar.activation(rs[:], lnms[:],
                                     mybir.ActivationFunctionType.Exp,
                                     bias=0.0, scale=-0.5)
                rsb = sb.tile([128, TT], f32, tag="rsb", name="rsb")
                nc.gpsimd.partition_broadcast(rsb[:], rs[:])
                # rotary, even dims on partitions 0:64, odd on 64:128:
                #   a      = q * fr            (both halves at once)
                #   bswap  = swap_halves(q) * [+fi; -fi]  (2 cross-half muls;
                #            the sign baked into fi makes the combine an add)
                #   rot    = a + bswap
                rot = sb.tile([128, TT], f32, tag="rot", name="rot")
                a = sb.tile([128, TT], f32, tag="rota", name="a")
                nc.vector.tensor_mul(a[:], ps[:], fr[:, ts])
                bsw = sb.tile([128, TT], f32, tag="rotb", name="bsw")
                nc.vector.tensor_mul(bsw[0:64, :], ps[64:128, :],
                                     fi[64:128, ts])
                nc.vector.tensor_mul(bsw[64:128, :], ps[0:64, :],
                                     fi[0:64, ts])
                nc.vector.tensor_add(rot[:], a[:], bsw[:])
                dst = krot if is_k else qrot[chunk]
                nc.vector.tensor_mul(dst[:, ts], rot[:], rsb[:])

        # ---------- phase 2 helper: one attention unit (head h, tile qt) ---
        def attention_unit(h, qt):
            ntk = 4 * (qt + 1)
            dps = prow.tile([1, TT], f32, tag="row", name="dps")
            yps = pacc.tile([128, TT], f32, tag="yacc", name="yps")
            for tk in range(ntk):
                # diagonal tiles (r >= 1) only have valid scores in their
                # last TT - 128*r columns; skip the fully-masked prefix.
                # In suffix-local coords the causal mask is always the r=0
                # triangle.
                r = tk - 4 * qt
                off = 128 * r if r > 0 else 0
                w = TT - off
                qs = slice(qt * TT + off, (qt + 1) * TT)
                sps = psum.tile([128, TT], f32, tag="mm", name="sps")
                nc.tensor.matmul(sps[:, :w],
                                 krot[:, tk * 128:(tk + 1) * 128],
                                 qrot[h][:, qs], start=True, stop=True)
                e = epool.tile([128, TT], bf16, tag="e", name="e")
                nc.scalar.activation(e[:, :w], sps[:, :w],
                                     mybir.ActivationFunctionType.Exp,
                                     bias=0.0, scale=SCALE)
                if r >= 0:
                    nc.vector.tensor_mul(e[:, :w], e[:, :w], cmask[:, :w])
                nc.tensor.matmul(dps[:, off:], ones[:], e[:, :w],
                                 start=(tk == 0), stop=(tk == ntk - 1))
                nc.tensor.matmul(yps[:, off:], vtok[:, tk, :], e[:, :w],
                                 start=(tk == 0), stop=(tk == ntk - 1))
            qs = slice(qt * TT, (qt + 1) * TT)
            dr = sb.tile([1, TT], f32, tag="dr", name="dr")
            nc.vector.reciprocal_approx_fast(dr[:], dps[:])
            drb = sb.tile([128, TT], f32, tag="drb", name="drb")
            nc.gpsimd.partition_broadcast(drb[:], dr[:])
            nc.vector.tensor_mul(yT[h][:, qs], yps[:], drb[:])

        # ---------- phase 3 helper: out-projection for one query tile ------
        def outproj(qt):
            qs = slice(qt * TT, (qt + 1) * TT)
            for m in range(D // 128):
                ops = pproj.tile([128, TT], f32, tag="proj", name="ops")
                for e in range(HEADS_PER_CORE):
                    nc.tensor.matmul(ops[:],
                                     woT[:, e, m * 128:(m + 1) * 128],
                                     yT[e][:, qs], start=(e == 0),
                                     stop=(e == HEADS_PER_CORE - 1))
                osb = sb.tile([128, TT], f32, tag="osb", name="osb")
                nc.vector.tensor_copy(osb[:], ops[:])
                nc.sync.dma_start(outT_d[m * 128:(m + 1) * 128, qs], osb[:])

        # ---------- emission: interleave attention with projection so PE
        # always has independent matmuls to issue while ACT runs exp -------
        project_chunk(HEADS_PER_CORE, wch=wch_next)    # k
        project_chunk(HEADS_PER_CORE + 1)              # v
        for i in range(N_SKT):                 # v -> [token, dv] layout
            tp = psum.tile([128, 128], bf16, tag="mm", name="tp")
            nc.tensor.transpose(tp[:], vT[:, i * 128:(i + 1) * 128], ident[:])
            nc.vector.tensor_copy(vtok[:, i, :], tp[:])
        for h in range(HEADS_PER_CORE):
            project_chunk(h)
        p1_ctx.close()   # xT/wqkvT/fr/fi no longer needed
        # largest query tiles first: their attention overlaps the remaining
        # projections.  The two smallest tiles are interleaved head-by-head
        # so the kernel tail has twice the independent chains to pipeline.
        for qt in (3, 2):
            for h in range(HEADS_PER_CORE):
                attention_unit(h, qt)
            # out-proj matmuls double as PE filler while ACT runs the next
            # tile's exps
            outproj(qt)
        for h in range(HEADS_PER_CORE):
            attention_unit(h, 1)
            attention_unit(h, 0)
        outproj(1)
        outproj(0)

    nc.compile()
    return nc


def _host_shards(x, freqs_cis, wqkv, wo):
    import ml_dtypes
    bf16 = ml_dtypes.bfloat16

    # head-dim permutation: even dims then odd dims (for q and k only)
    perm = np.concatenate([np.arange(0, HD, 2), np.arange(1, HD, 2)])

    wq = wqkv[:Q_SIZE].reshape(NH, HD, D)[:, perm, :]
    wk = wqkv[Q_SIZE:Q_SIZE + KV_SIZE].reshape(NKV, HD, D)[:, perm, :]
    wv = wqkv[Q_SIZE + KV_SIZE:].reshape(NKV, HD, D)

    fr1 = np.ascontiguousarray(freqs_cis[:, :, 0].T, dtype=np.float32)
    fi1 = np.ascontiguousarray(freqs_cis[:, :, 1].T, dtype=np.float32)
    fr = np.vstack([fr1, fr1])
    # sign baked in so the rotary combine is a single add:
    #   rot[lo] = q_lo*fr + q_hi*(-fi) ; rot[hi] = q_hi*fr + q_lo*(+fi)
    fi = np.vstack([fi1, -fi1])

    # causal mask for the leading diagonal of a 128-row x 512-col score
    # tile (suffix-narrowed diagonal tiles reuse its prefix columns)
    tkl = np.arange(128)[:, None]
    tql = np.arange(TT)[None, :]
    mask = (tkl <= tql).astype(bf16)

    in_maps = []
    for c in range(N_CORES):
        b, j = divmod(c, TPC)
        wshard = np.concatenate(
            [wq[TPC * j + h] for h in range(HEADS_PER_CORE)] +
            [wk[j], wv[j]], axis=0)                     # (768, D)
        # [chunk, p, ko, e] with d = ko*128 + p
        wpack = np.ascontiguousarray(
            wshard.reshape(HEADS_PER_CORE + 2, HD, N_KT, 128)
            .transpose(0, 3, 2, 1)).astype(bf16)
        in_maps.append({
            "xT": np.ascontiguousarray(x[b].T).astype(bf16),
            "wqkvT": wpack,
            "woT": np.ascontiguousarray(
                wo[:, j * E_LOC:(j + 1) * E_LOC].T).astype(bf16),
            "fr": fr,
            "fi": fi,
            "mask": mask,
        })
    return in_maps


_NC_CACHE = {}


def _get_nc():
    if "nc" not in _NC_CACHE:
        _NC_CACHE["nc"] = _build_bass()
    return _NC_CACHE["nc"]


def kernel(x, freqs_cis, wqkv, wo, q_norm_w, k_norm_w, _want_results=False):
    # q_norm_w / k_norm_w are all-ones per the problem spec; rmsnorm weight
    # multiply is the identity and is folded away.
    from concourse.bass_utils import run_bass_kernel_spmd

    nc = _get_nc()
    in_maps = _host_shards(np.asarray(x, np.float32),
                           np.asarray(freqs_cis, np.float32),
                           np.asarray(wqkv, np.float32),
                           np.asarray(wo, np.float32))
    res = run_bass_kernel_spmd(nc, in_maps, core_ids=list(range(N_CORES)))
    parts = [r["outT"] for r in res.results]
    out = np.empty((B, S, D), np.float32)
    for b in range(B):
        acc = parts[TPC * b].astype(np.float32).copy()
        for j in range(1, TPC):
            acc += parts[TPC * b + j]
        out[b] = acc.T
    if _want_results:
        return out, res
    return out
